# revision 62
# baseline (speedup 1.0000x reference)
"""Trainium2 Bass kernel for nn_BasicCGInducer (CKY inside algorithm for a
categorial-grammar inducer).

Strategy (8 NeuronCores):
  - Data-parallel over sentences: core j handles sentences 4j..4j+3.
  - Emission log-partition (the big [C,V] softmax denominator) is
    tensor-parallel over vocab: each core computes sum_v exp(logits) for a
    4000-column V-shard, then one AllReduce of [C] partial sums.
  - Everything else (grammar tables, split-MLP, beta1, CKY) is computed
    per-core on its sentence shard in scaled-exp space (no logsumexp on the
    hot path; per-span running max scales).

kernel(**inputs) takes FULL inputs, shards on host, runs one SPMD bass
program on cores 0-7, and reassembles the [32] output.
"""
import sys
import contextlib

sys.path.insert(0, "/opt/trn_rl_repo")

import numpy as np

import concourse.bass as bass
import concourse.bacc as bacc
import concourse.mybir as mybir
import concourse.tile as tile
from concourse.ap import AP
from concourse import bass_utils

F32 = mybir.dt.float32
F32R = mybir.dt.float32r
BF16 = mybir.dt.bfloat16
I32 = mybir.dt.int32
F8E4 = mybir.dt.float8e4
ALU = mybir.AluOpType
ACTF = mybir.ActivationFunctionType
AXIS = mybir.AxisListType
LN2 = 0.6931471805599453
FE_A = 12102203.161561485           # 2^23 / ln 2
FE_B = float((127 << 23) - 486411)  # Schraudolph bias, rms-centred

# ---------------------------------------------------------------- constants
P4 = 4          # primitive cats
NF = 36         # non-functor cats
C = 2596        # total cats
CP = 2688       # padded C (21 * 128)
NT = CP // 128  # 21 c-tiles
D = 64
B = 32          # total sentences
NCORES = 8
BLOC = B // NCORES  # 4 sentences per core
V = 32000
BLK2 = 72       # per-level block stride in bf16 chart tensors
NEGB = -1.0e5   # bias for padded vocab columns


class Cfg:
    def __init__(self, n=32, v_loc=4000, n_cores=8):
        self.n = n                      # sentence length
        self.v_loc = v_loc              # vocab shard per core
        self.v_pad = ((v_loc + 511) // 512) * 512
        self.n_cores = n_cores
        self.pairs = 4 * n              # (i, b) pairs on partitions


# ------------------------------------------------------------ functor maps
def lf_block_offsets(op):
    """c = off + {A: 4r+a | B: 32r+(a-4) | C: 36(r-4)+a} per derivation of
    the deterministic functor-id tables. op=0 -> l_functors, 1 -> r_functors."""
    return {
        "A": 4 + 16 * op,            # res<4, arg<4 : c = A + 4*res + arg
        "B": 36 + 1280 * op,         # res<4, arg>=4: c = B + 32*res + (arg-4)
        "C": 164 + 1280 * op,        # res>=4      : c = C0 + 36*(res-4) + arg
    }


def check_functor_tables(l_functors, r_functors):
    for op, tab in ((0, l_functors), (1, r_functors)):
        off = lf_block_offsets(op)
        exp = np.zeros((NF, NF), np.int64)  # [arg, res]
        for res in range(NF):
            for arg in range(NF):
                if res < P4 and arg < P4:
                    exp[arg, res] = off["A"] + 4 * res + arg
                elif res < P4:
                    exp[arg, res] = off["B"] + 32 * res + (arg - 4)
                else:
                    exp[arg, res] = off["C"] + 36 * (res - 4) + arg
        assert np.array_equal(np.asarray(tab, np.int64), exp), (
            f"functor table structure mismatch (op={op})")


# ---------------------------------------------------------------- AP helper
def mk(t, parts, off, dims, base_part=0):
    """Raw AP on tile t: partition range [base_part, base_part+parts),
    free offset `off` (elements), extra free dims [[step, count], ...]."""
    w = t.ap[0][0]
    return AP(t.tensor, t.offset + base_part * w + off, [[w, parts]] + dims)


def mkS(t, parts, off, blocks=1, step=72, base_part=0):
    """fp32 view of a pair of bf16 cols at `off` (+k*step) in bf16 tile t."""
    w = t.ap[0][0]
    ap = AP(t.tensor, t.offset + base_part * w + off,
            [[w, parts], [step, blocks], [1, 2]])
    return ap.bitcast(mybir.dt.float32)


# ============================================================ device program
def build_program(cfg: Cfg):
    nc = bacc.Bacc("TRN2", target_bir_lowering=False, debug=False,
                   num_devices=cfg.n_cores)
    d = {
        "ntembT": nc.dram_tensor("ntembT", [65, CP], BF16,
                                 kind="ExternalInput"),
        "ntembT8": nc.dram_tensor("ntembT8", [33, 2 * CP], F8E4,
                                  kind="ExternalInput"),
        "vocabW8": nc.dram_tensor("vocabW8", [33, 2 * cfg.v_pad], F8E4,
                                  kind="ExternalInput"),
        "wordW": nc.dram_tensor("wordW", [65, cfg.pairs], BF16,
                                kind="ExternalInput"),
        "mlpW": nc.dram_tensor("mlpW", [64, 322], BF16, kind="ExternalInput"),
        "mlpB": nc.dram_tensor("mlpB", [64, 8], F32, kind="ExternalInput"),
        "ruleWb": nc.dram_tensor("ruleWb", [36, 144], F32,
                                 kind="ExternalInput"),
        "smallv": nc.dram_tensor("smallv", [1, 16], F32,
                                 kind="ExternalInput"),
        "out": nc.dram_tensor("out_nll", [BLOC, 1], F32,
                              kind="ExternalOutput"),
    }
    with tile.TileContext(nc) as tc:
        _trace(tc, cfg, d)
    nc.compile()
    return nc


def _trace(tc, cfg, d):
    nc = tc.nc
    n, PAIRS, VP = cfg.n, cfg.pairs, cfg.v_pad
    NV = VP // 512                    # 512-col v-tiles per core
    NHALF = (NV + 3) // 4             # ACT chunks of up to 4 v-tiles
    HW = CP // 2                      # MLP half width (1344)

    es = contextlib.ExitStack()
    keep = es.enter_context(tc.tile_pool(name="keep", bufs=1))
    dram = es.enter_context(tc.tile_pool(name="dram", bufs=1, space="DRAM"))

    # ---------------- long-lived tensors
    # chart blocks (bf16 values): 0:36 inside | 36:52 FA | 52:68 FB | pad 4
    chartV = keep.tile([PAIRS, (n + 1) * BLK2], BF16)
    chartEV = keep.tile([PAIRS, (n + 1) * BLK2], BF16)  # end-indexed, rev
    WA = keep.tile([PAIRS, 1300], BF16)   # cols 1296:1298 = M1 (f32 bits)
    WB = keep.tile([PAIRS, 1300], BF16)
    grgl = keep.tile([128, 2592], BF16)   # Gr at 0:1296, Gl at 1296:2592
    M1 = keep.tile([PAIRS, 2], F32)
    mlpB = keep.tile([64, 8], F32)
    smallv = keep.tile([1, 16], F32)
    sumexp_parts = keep.tile([128, NT * NHALF], F32)
    sumexp_loc = keep.tile([128, NT], F32)
    sumexp_g = keep.tile([128, NT], F32)
    s0E = keep.tile([1, NF], F32)
    db = keep.tile([1, 2], F32)
    rsRep = keep.tile([4, 4], F32)
    fin = keep.tile([4, 8], F32)

    nc.sync.dma_start(mlpB[:], d["mlpB"][:])
    nc.sync.dma_start(smallv[:], d["smallv"][:])
    nc.gpsimd.memset(chartV[:], 0.0)
    nc.gpsimd.memset(chartEV[:], 0.0)

    ph1 = contextlib.ExitStack()
    p1 = ph1.enter_context(tc.tile_pool(name="ph1", bufs=1))
    ntembT = p1.tile([65, CP], BF16)
    ntembT8 = p1.tile([33, 2 * CP], F8E4)
    vocabW8 = p1.tile([33, 2 * VP], F8E4)
    wordW = p1.tile([65, PAIRS], BF16)
    mlpW = p1.tile([64, 322], BF16)
    ruleWb = p1.tile([36, 144], F32)
    adjE = p1.tile([1, CP], F32)      # exp-space split1 factor sigmoid(-y)
    zrec_row = p1.tile([1, CP], F32)  # 1/Z per cat, flattened
    E_row = p1.tile([1, CP], F32)     # sigmoid(-y)/Z
    E_bf = p1.tile([1, CP], BF16)
    Erep = p1.tile([PAIRS, CP], BF16)
    zrec21 = p1.tile([128, NT], F32)
    beta1E = p1.tile([PAIRS, CP], BF16)
    ruleflat = p1.tile([1, 36 * 72], F32)

    nc.sync.dma_start(ntembT[:], d["ntembT"][:])
    nc.sync.dma_start(ntembT8[:], d["ntembT8"][:])
    nc.sync.dma_start(vocabW8[:], d["vocabW8"][:])
    nc.sync.dma_start(wordW[:], d["wordW"][:])
    nc.sync.dma_start(mlpW[:], d["mlpW"][:])
    nc.sync.dma_start(ruleWb[:], d["ruleWb"][:])

    # =======================================================================
    # Phase 1: emission partition function (exp in place in PSUM + accum_out)
    # =======================================================================
    # AllReduce is split in two halves of c-tiles: the first is issued as
    # soon as tiles 0..NTH-1 finish, hiding its latency under the tail of
    # the emission loop.
    NTH = 14                 # tiles in cc half 1
    cc_in1 = dram.tile([128, NTH], F32)
    cc_out1 = dram.tile([128, NTH], F32)
    cc_in2 = dram.tile([128, NT - NTH], F32)
    cc_out2 = dram.tile([128, NT - NTH], F32)
    cc_bufs = {0: (cc_in1, cc_out1), NTH: (cc_in2, cc_out2)}
    rg = [list(range(cfg.n_cores))]

    def cc_half(r0, r1):
        ci, co = cc_bufs[r0]
        nc.vector.tensor_reduce(
            sumexp_loc[:, r0:r1],
            mk(sumexp_parts, 128, r0 * NHALF, [[NHALF, r1 - r0], [1, NHALF]]),
            axis=AXIS.X, op=ALU.add)
        nc.sync.dma_start(ci[:], sumexp_loc[:, r0:r1])
        nc.gpsimd.collective_compute(
            "AllReduce", ALU.add, replica_groups=rg,
            ins=[ci[:].opt()], outs=[co[:].opt()])

    with tc.tile_pool(name="psum_e", bufs=2, space="PSUM") as pse, \
         tc.tile_pool(name="scr_e", bufs=2) as scre:
        for ct in range(NT):
            for h in range(NHALF):
                vt0 = h * 4
                nvt = min(4, NV - vt0)
                idx = ct * NHALF + h
                c_lo = vt0 * 512
                c_hi = min((vt0 + nvt) * 512, cfg.v_loc)  # skip pad columns
                pt = pse.tile([128, 512 * nvt], F32, tag="pse")
                for vt in range(nvt):
                    w0 = vt * 512
                    w1 = min((vt + 1) * 512, c_hi - c_lo)
                    if w1 <= w0:
                        continue
                    nc.tensor.matmul(
                        pt[:, w0:w1],
                        mk(ntembT8, 33, ct * 128, [[CP, 2], [1, 128]]),
                        mk(vocabW8, 33, c_lo + w0, [[VP, 2], [1, w1 - w0]]),
                        start=True, stop=True,
                        perf_mode=mybir.MatmulPerfMode.DoubleRow)
                sce = scre.tile([128, 512 * 4], BF16, tag="scre")
                nc.scalar.activation(
                    sce[:, 0:c_hi - c_lo], pt[:, 0:c_hi - c_lo], ACTF.Exp,
                    accum_out=sumexp_parts[:, idx:idx + 1])
            if ct == NTH - 1:
                cc_half(0, NTH)
    cc_half(NTH, NT)

    # =======================================================================
    # Phase 2: split MLP (transposed layout hT [64, *]), rule tables, root
    # (independent of the AllReduce -> overlaps it)
    # =======================================================================
    nc.vector.tensor_tensor(db[:, 0:1], smallv[:, 0:1], smallv[:, 1:2],
                            op=ALU.subtract)

    with tc.tile_pool(name="mlp", bufs=1) as mlp:
        hA = mlp.tile([64, HW], BF16, tag="hA")
        hB = mlp.tile([64, HW], BF16, tag="hB")
        hC = mlp.tile([64, HW], BF16, tag="hC")
        s_rows = mlp.tile([2, HW], F32, tag="srows")
        w1 = mlp.tile([1, HW], F32, tag="w1")
        w2 = mlp.tile([1, HW], F32, tag="w2")
        w3 = mlp.tile([1, HW], F32, tag="w3")

        for half in range(2):
            base = half * HW

            def dense_relu(dst, col0, rhs, bias_col, res_add=None, rb=0,
                           func=ACTF.Relu):
                with tc.tile_pool(name="psum_m", bufs=2,
                                  space="PSUM") as psm:
                    for c0 in range(0, HW, 512):
                        c1 = min(c0 + 512, HW)
                        pm = psm.tile([64, 512], F32, tag="psm")
                        nc.tensor.matmul(pm[:, 0:c1 - c0],
                                         mlpW[:, col0:col0 + 64],
                                         rhs[0:64, rb + c0:rb + c1],
                                         start=True, stop=True)
                        nc.scalar.activation(
                            dst[:, c0:c1], pm[:, 0:c1 - c0], func,
                            bias=mlpB[:, bias_col:bias_col + 1])
                        if res_add is not None:
                            nc.vector.tensor_tensor(
                                dst[:, c0:c1], dst[:, c0:c1],
                                res_add[:, c0:c1], op=ALU.add)

            dense_relu(hA, 0, ntembT, 0, rb=base,
                       func=ACTF.Identity)           # h1 (linear)
            dense_relu(hB, 64, hA, 1)                   # t = relu(h1 W + b)
            dense_relu(hC, 128, hB, 2, res_add=hA)      # h2
            dense_relu(hB, 192, hC, 3)                  # t2
            dense_relu(hA, 256, hB, 4, res_add=hC)      # h3

            with tc.tile_pool(name="psum_s", bufs=2, space="PSUM") as pss:
                for c0 in range(0, HW, 512):
                    c1 = min(c0 + 512, HW)
                    ps = pss.tile([2, 512], F32, tag="pss")
                    nc.tensor.matmul(ps[:, 0:c1 - c0],
                                     mlpW[:, 320:322],
                                     hA[0:64, c0:c1],
                                     start=True, stop=True)
                    nc.vector.tensor_copy(s_rows[:, c0:c1], ps[:, 0:c1 - c0])

            # d = s0 - s1 (s1 via DMA to partition 0)
            nc.sync.dma_start(w1[:], s_rows[1:2, :])
            nc.vector.tensor_tensor(w2[:], s_rows[0:1, :], w1[:],
                                    op=ALU.subtract)
            y = w2
            nc.vector.tensor_scalar_add(y[:], y[:], db[:, 0:1])
            # exp(split1) = exp(-softplus(y)) = sigmoid(-y)
            nc.scalar.activation(adjE[:, base:base + HW], y[:],
                                 ACTF.Sigmoid, scale=-1.0)
            if half == 0:
                # exp(split0) = exp(-softplus(-y)) = sigmoid(y)
                nc.scalar.activation(s0E[:], y[:, 0:NF], ACTF.Sigmoid)

    # rule tables: softmax over 72 per res row
    rsum = keep.tile([36, 72], F32)
    rmax = keep.tile([36, 2], F32)
    rsumexp = keep.tile([36, 2], F32)
    nc.vector.tensor_tensor(rsum[:], ruleWb[:, 0:72], ruleWb[:, 72:144],
                            op=ALU.add)
    nc.vector.tensor_reduce(rmax[:, 0:1], rsum[:], axis=AXIS.X, op=ALU.max)
    nc.vector.tensor_scalar_mul(rmax[:, 1:2], rmax[:, 0:1], -1.0)
    nc.scalar.activation(rsum[:], rsum[:], ACTF.Exp, bias=rmax[:, 1:2],
                         accum_out=rsumexp[:, 0:1])
    nc.vector.reciprocal(rsumexp[:, 1:2], rsumexp[:, 0:1])
    nc.vector.tensor_scalar_mul(rsum[:], rsum[:], rsumexp[:, 1:2])

    # flatten ruleEn to [1, 2592] via DRAM, then G-flats replicated
    rule_d = dram.tile([36, 72], F32)
    nc.sync.dma_start(rule_d[:], rsum[:])
    nc.sync.dma_start(
        AP(ruleflat.tensor, ruleflat.offset,
           [[ruleflat.ap[0][0], 1], [1, 36 * 72]]),
        rule_d[:])
    g_d = dram.tile([2, 1296], BF16)
    gtmp = keep.tile([1, 1296], BF16)
    for row, off in ((0, 0), (1, 36)):   # 0: Gl (larg), 1: Gr (rarg)
        nc.vector.tensor_tensor(
            gtmp[:],
            mk(ruleflat, 1, off, [[72, 36], [1, 36]]),
            mk(s0E, 1, 0, [[1, 36], [0, 36]]),
            op=ALU.mult)
        nc.sync.dma_start(g_d[row:row + 1, :], gtmp[:])
    for goff, row in ((1296, 0), (0, 1)):   # row0=Gl -> cols 1296+, row1=Gr
        nc.sync.dma_start(
            grgl[:, goff:goff + 1296],
            AP(g_d.tensor, g_d.offset + row * g_d.ap[0][0],
               [[0, 128], [1, 1296]]))

    # root: rsEn = softmax(root_W[0,0:4] + root_b[0:4]) replicated to 4 parts
    rs4 = keep.tile([1, 8], F32)
    rsE = keep.tile([1, 8], F32)
    nc.vector.tensor_tensor(rs4[:, 0:4], smallv[:, 2:6], smallv[:, 6:10],
                            op=ALU.add)
    nc.vector.tensor_reduce(rs4[:, 4:5], rs4[:, 0:4], axis=AXIS.X, op=ALU.max)
    nc.vector.tensor_scalar_mul(rs4[:, 5:6], rs4[:, 4:5], -1.0)
    nc.scalar.activation(rsE[:, 0:4], rs4[:, 0:4], ACTF.Exp,
                         bias=rs4[:, 5:6], accum_out=rsE[:, 4:5])
    nc.vector.reciprocal(rsE[:, 5:6], rsE[:, 4:5])
    nc.vector.tensor_scalar_mul(rsE[:, 0:4], rsE[:, 0:4], rsE[:, 5:6])
    rs_d = dram.tile([1, 4], F32)
    nc.sync.dma_start(rs_d[:], rsE[:, 0:4])
    nc.sync.dma_start(rsRep[:],
                      AP(rs_d.tensor, rs_d.offset, [[0, 4], [1, 4]]))

    # =======================================================================
    # Phase 3: beta1 = wordW.T @ ntembT (no adj row; E-factor applied after
    # the AllReduce lands) -> exp tables WA/WB
    # =======================================================================
    with tc.tile_pool(name="psum_b", bufs=1, space="PSUM") as psb:
        pb = psb.tile([PAIRS, CP], F32)
        for c0 in range(0, CP, 512):
            c1 = min(c0 + 512, CP)
            nc.tensor.matmul(pb[:, c0:c1], wordW[:],
                             ntembT[:, c0:c1],
                             start=True, stop=True)
        nc.vector.tensor_reduce(M1[:, 0:1], pb[:, 0:C], axis=AXIS.X,
                                op=ALU.max)
        nc.vector.tensor_scalar_mul(M1[:, 1:2], M1[:, 0:1], -1.0)
        nc.scalar.activation(beta1E[:], pb[:], ACTF.Exp, bias=M1[:, 1:2])

    # ---- AllReduce-dependent tail: E[c] = sigmoid(-y_c) / Z_c, per cc half
    # (half 1 lands while the emission tail is still running)
    z_d = dram.tile([128, NT], F32)
    e_d = dram.tile([1, CP], BF16)
    def e_half(r0, r1):
        c0, c1 = r0 * 128, r1 * 128
        nc.sync.dma_start(sumexp_g[:, r0:r1], cc_bufs[r0][1][:])
        nc.vector.reciprocal(zrec21[:, r0:r1], sumexp_g[:, r0:r1])
        # rearrange [128, r] -> [1, c]  (c = ct*128 + p) via DRAM bounce
        nc.sync.dma_start(z_d[:, r0:r1], zrec21[:, r0:r1])
        nc.sync.dma_start(
            AP(zrec_row.tensor, zrec_row.offset + c0,
               [[zrec_row.ap[0][0], 1], [128, r1 - r0], [1, 128]]),
            AP(z_d.tensor, z_d.offset + r0,
               [[z_d.ap[0][0], 1], [1, r1 - r0], [NT, 128]]))
        nc.vector.tensor_tensor(E_bf[:, c0:c1], adjE[:, c0:c1],
                                zrec_row[:, c0:c1], op=ALU.mult)
        nc.sync.dma_start(e_d[:, c0:c1], E_bf[:, c0:c1])
        nc.sync.dma_start(
            Erep[:, c0:c1],
            AP(e_d.tensor, e_d.offset + c0, [[0, PAIRS], [1, c1 - c0]]))
        nc.vector.tensor_tensor(beta1E[:, c0:c1], beta1E[:, c0:c1],
                                Erep[:, c0:c1], op=ALU.mult)

    def w_block(W, goff, oo, od, io, idm):
        nc.vector.scalar_tensor_tensor(
            mk(W, PAIRS, oo, od), mk(beta1E, PAIRS, io, idm), 1.0,
            mk(grgl, PAIRS, goff + oo, od), op0=ALU.mult, op1=ALU.mult)

    offB = lf_block_offsets(0)   # WB gathers l_functors
    offA = lf_block_offsets(1)   # WA gathers r_functors
    e_half(0, NTH)
    # everything whose beta1E sources lie in cats < NTH*128 = 2176 runs now,
    # hidden under the emission tail / second AllReduce
    w_block(WB, 0, 0, [[36, 4], [1, 4]], offB["A"], [[4, 4], [1, 4]])
    w_block(WB, 0, 4, [[36, 4], [1, 32]], offB["B"], [[32, 4], [1, 32]])
    w_block(WB, 0, 144, [[1, 1152]], offB["C"], [[1, 1152]])
    w_block(WA, 1296, 0, [[36, 4], [1, 4]], offA["A"], [[4, 4], [1, 4]])
    w_block(WA, 1296, 4, [[36, 4], [1, 32]], offA["B"], [[32, 4], [1, 32]])
    cs = NTH * 128 - offA["C"]   # C-block cols with source cat < NTH*128
    w_block(WA, 1296, 144, [[1, cs]], offA["C"], [[1, cs]])
    nc.vector.tensor_copy(mkS(WA, PAIRS, 1296), M1[:, 0:1])
    # chart block L=1 from beta1E (cats < 36, all in half 1)
    nc.vector.tensor_copy(mk(chartV, PAIRS, BLK2, [[1, 36]]), beta1E[:, 0:NF])
    nc.vector.tensor_copy(mkS(chartV, PAIRS, BLK2 + 68), M1[:, 0:1])
    nc.vector.tensor_tensor(
        mk(chartV, PAIRS, BLK2 + 36, [[16, 2], [4, 4], [1, 4]]),
        mk(beta1E, PAIRS, 4, [[16, 2], [4, 4], [1, 4]]),
        mk(grgl, PAIRS, 0, [[1296, 2], [36, 4], [1, 4]]),
        op=ALU.mult)
    # chartEnd block m lives at col (n-m)*BLK2 (reversed layout; makes all
    # k-strided reads positive-step). Block 1: end j = i+1 -> same rows.
    nc.sync.dma_start(mk(chartEV, PAIRS, (n - 1) * BLK2, [[1, BLK2]]),
                      mk(chartV, PAIRS, BLK2, [[1, BLK2]]))
    e_half(NTH, NT)
    w_block(WA, 1296, 144 + cs, [[1, 1152 - cs]],
            offA["C"] + cs, [[1, 1152 - cs]])

    ph1.close()  # free ph1 tensors before the CKY working set

    es2 = contextlib.ExitStack()
    stage_pool = es2.enter_context(tc.tile_pool(name="stage", bufs=4))
    wash_pool = es2.enter_context(tc.tile_pool(name="wash", bufs=4))
    scr = es2.enter_context(tc.tile_pool(name="cky", bufs=4))
    scr1 = es2.enter_context(tc.tile_pool(name="cky1", bufs=3))

    # =======================================================================
    # Phase 4: CKY in scaled-exp space (bf16 values, fp32 scales)
    # chartV[pair, L*BLK2+.]: 0:36 inside | 36:52 FA | 52:68 FB
    # scale (f32 in bf16 cols 68:70). chartEV end-indexed by span end j,
    # block m at col (n-m)*BLK2.
    # =======================================================================
    NI_MAX = max(n - 2, 1)
    for L in range(2, n + 1):
        S = n - L + 1
        PS = 4 * S
        NI = L - 2

        stageV = stage_pool.tile([128, n * BLK2], BF16, tag="stv")
        if L >= 3:   # prefetchable part: blocks 1..L-2 (scalar DMA queue
            # to balance against sync's chartEV-write + wash)
            nc.scalar.dma_start(
                mk(stageV, PS, (n - L + 2) * BLK2, [[1, (L - 2) * BLK2]]),
                mk(chartEV, PS, (n - L + 2) * BLK2, [[1, (L - 2) * BLK2]],
                   base_part=4 * (L - 1)))
        # critical part: block L-1 = chartV rows [4 .. 4+PS]; issued on the
        # scalar engine's DMA path so it skips the sync queue backlog
        nc.scalar.dma_start(
            mk(stageV, PS, (n - L + 1) * BLK2, [[1, BLK2]]),
            mk(chartV, PS, (L - 1) * BLK2, [[1, BLK2]], base_part=4))

        wash = wash_pool.tile([128, 1300], BF16, tag="wa")
        nc.sync.dma_start(
            mk(wash, PS, 0, [[1, 1300]]),
            mk(WA, PS, 0, [[1, 1300]], base_part=4 * (L - 1)))

        # ---- edge products (eA/eB scalars factored out so prodA has no
        # DMA or scale dependency and starts the round immediately; the
        # sAsm/mstar/exp chain runs concurrently between the two products).
        # The final round only needs the root's primitive results (res<4).
        RS = 4 if L == n else 36
        prodAB = scr1.tile([128, 2592], BF16, tag="prod")
        nc.vector.tensor_tensor(
            prodAB[0:PS, 0:RS * 36],
            mk(wash, PS, 0, [[1, RS * 36]]),
            mk(chartV, PS, (L - 1) * BLK2, [[0, RS], [1, 36]]),
            op=ALU.mult)
        # ---- scales: sAsm = [sB | sA | sI(k=1..L-1)] -- runs while prodA
        # streams; eAll is only needed after the reduces
        sAsm = scr.tile([128, n + 8], F32, tag="sasm")
        nc.vector.tensor_tensor(
            sAsm[0:PS, 0:1], mkS(stageV, PS, (n - L + 1) * BLK2 + 68),
            M1[0:PS, 0:1], op=ALU.add)
        nc.vector.tensor_tensor(
            sAsm[0:PS, 1:2], mkS(chartV, PS, (L - 1) * BLK2 + 68),
            mkS(wash, PS, 1296), op=ALU.add)
        nc.vector.tensor_tensor(
            sAsm[0:PS, 2:L + 1],
            mkS(chartV, PS, BLK2 + 68, blocks=L - 1),
            mkS(stageV, PS, (n - L + 1) * BLK2 + 68, blocks=L - 1),
            op=ALU.add)
        mstar = scr.tile([128, 2], F32, tag="mstar")
        nc.vector.tensor_reduce(mstar[0:PS, 1:2], sAsm[0:PS, 0:L + 1],
                                axis=AXIS.X, op=ALU.max, negate=True)
        eAll = scr.tile([128, n + 8], F32, tag="eall")
        nc.scalar.activation(eAll[0:PS, 0:L + 1], sAsm[0:PS, 0:L + 1],
                             ACTF.Exp, bias=mstar[0:PS, 1:2])
        nc.vector.tensor_tensor(
            prodAB[0:PS, 1296:1296 + RS * 36],
            mk(WB, PS, 0, [[1, RS * 36]]),
            mk(stageV, PS, (n - L + 1) * BLK2, [[0, RS], [1, 36]]),
            op=ALU.mult)
        redAB = scr.tile([128, 72], F32, tag="red")
        nc.vector.tensor_reduce(redAB[0:PS, 0:RS],
                                mk(prodAB, PS, 0, [[36, RS], [1, 36]]),
                                axis=AXIS.X, op=ALU.add)
        nc.vector.tensor_reduce(redAB[0:PS, RS:2 * RS],
                                mk(prodAB, PS, 1296, [[36, RS], [1, 36]]),
                                axis=AXIS.X, op=ALU.add)
        total36 = scr.tile([128, 40], F32, tag="tot")
        nc.vector.tensor_scalar_mul(total36[0:PS, 0:RS],
                                    redAB[0:PS, 0:RS], eAll[0:PS, 1:2])
        nc.vector.scalar_tensor_tensor(
            total36[0:PS, 0:RS], redAB[0:PS, RS:2 * RS], eAll[0:PS, 0:1],
            total36[0:PS, 0:RS], op0=ALU.mult, op1=ALU.add)


        # ---- interior terms (res<4), batched over k, eI pre-folded in args
        if NI > 0:
            argsI = scr.tile([128, 8 * NI_MAX], BF16, tag="argsi")
            nc.vector.tensor_tensor(   # left args (chart k) x eI[k]
                mk(argsI, PS, 0, [[4, NI], [1, 4]]),
                mk(chartV, PS, BLK2, [[BLK2, NI], [1, 4]]),
                mk(eAll, PS, 2, [[1, NI], [0, 4]]), op=ALU.mult)
            nc.vector.tensor_tensor(   # right args (stage L-k) x eI[k]
                mk(argsI, PS, 4 * NI_MAX, [[4, NI], [1, 4]]),
                mk(stageV, PS, (n - L + 2) * BLK2, [[BLK2, NI], [1, 4]]),
                mk(eAll, PS, 3, [[1, NI], [0, 4]]), op=ALU.mult)
            # tI in k-major 16-blocks (j = 4*res+arg) so the fills read the
            # contiguous FA/FB chart blocks with flat 2-level patterns
            tI = scr1.tile([128, 2 * NI_MAX * 16], BF16, tag="ti")
            nc.vector.tensor_tensor(   # IA: scaled left args x stage FA(L-k)
                mk(tI, PS, 0, [[16, NI], [1, 16]]),
                mk(argsI, PS, 0, [[4, NI], [0, 4], [1, 4]]),
                mk(stageV, PS, (n - L + 1) * BLK2 + 52, [[BLK2, NI], [1, 16]]),
                op=ALU.mult)
            nc.vector.tensor_tensor(   # IB: scaled right args x chart[k] FB
                mk(tI, PS, 16 * NI, [[16, NI], [1, 16]]),
                mk(argsI, PS, 4 * NI_MAX, [[4, NI], [0, 4], [1, 4]]),
                mk(chartV, PS, 2 * BLK2 + 36, [[BLK2, NI], [1, 16]]),
                op=ALU.mult)
            nc.vector.tensor_reduce(   # sum over (side*k, arg) -> [PS, 4]
                total36[0:PS, 36:40],
                mk(tI, PS, 0, [[4, 4], [16, 2 * NI], [1, 4]]),
                axis=AXIS.XY, op=ALU.add)
            nc.vector.tensor_tensor(total36[0:PS, 0:4], total36[0:PS, 0:4],
                                    total36[0:PS, 36:40], op=ALU.add)

        # ---- rescale by a power of 2 near the max (log2-exponent bit
        # tricks on DVE; keeps Exp as the only scalar-engine table in CKY).
        # Odd rounds skip the renorm: values stay bounded enough for one
        # extra level and the next even round rescales.
        if L % 2 == 0 or L == n:
            mval = scr.tile([128, 8], F32, tag="mval")
            nc.vector.tensor_reduce(mval[0:PS, 0:1], total36[0:PS, 0:RS],
                                    axis=AXIS.X, op=ALU.max)
            nc.vector.tensor_scalar(
                mval[0:PS, 5:6].bitcast(I32), mval[0:PS, 0:1].bitcast(I32),
                0x7F800000, None, op0=ALU.bitwise_and)      # exponent bits
            nc.vector.tensor_scalar(
                mval[0:PS, 6:7].bitcast(I32), mval[0:PS, 5:6].bitcast(I32),
                -1, 0x7F000000, op0=ALU.mult, op1=ALU.add)  # bits 2^(127-e)
            nc.vector.tensor_scalar(
                mval[0:PS, 4:5].bitcast(I32), mval[0:PS, 0:1].bitcast(I32),
                23, None, op0=ALU.logical_shift_right)      # biased exp e
            nc.vector.tensor_scalar(
                mval[0:PS, 3:4], mval[0:PS, 4:5].bitcast(I32),
                127, None, op0=ALU.subtract)                # e - 127 (as f32)
            nc.vector.tensor_scalar_mul(
                mk(chartV, PS, L * BLK2, [[1, RS]]),
                total36[0:PS, 0:RS], mval[0:PS, 6:7])
            nc.vector.scalar_tensor_tensor(
                mkS(chartV, PS, L * BLK2 + 68),
                mval[0:PS, 3:4], LN2, mstar[0:PS, 1:2],
                op0=ALU.mult, op1=ALU.subtract)
        else:
            nc.vector.tensor_copy(
                mk(chartV, PS, L * BLK2, [[1, 36]]), total36[0:PS, 0:36])
            nc.vector.tensor_scalar_mul(
                mkS(chartV, PS, L * BLK2 + 68), mstar[0:PS, 1:2], -1.0)
        if L < n:
            nc.vector.tensor_tensor(
                mk(chartV, PS, L * BLK2 + 36, [[16, 2], [4, 4], [1, 4]]),
                mk(chartV, PS, L * BLK2 + 4, [[16, 2], [4, 4], [1, 4]]),
                mk(grgl, PS, 0, [[1296, 2], [36, 4], [1, 4]]),
                op=ALU.mult)
        if L < n:   # chartEnd block L at rows (i+L-1)*4+b
            nc.sync.dma_start(
                mk(chartEV, PS, (n - L) * BLK2, [[1, BLK2]],
                   base_part=4 * (L - 1)),
                mk(chartV, PS, L * BLK2, [[1, BLK2]]))

    # =======================================================================
    # Phase 5: root -> nll per sentence
    # =======================================================================
    nc.vector.tensor_tensor(fin[:, 0:4],
                            mk(chartV, 4, n * BLK2, [[1, 4]]),
                            rsRep[:], op=ALU.mult)
    nc.vector.tensor_reduce(fin[:, 4:5], fin[:, 0:4], axis=AXIS.X, op=ALU.add)
    nc.scalar.activation(fin[:, 5:6], fin[:, 4:5], ACTF.Ln)
    nc.vector.scalar_tensor_tensor(
        fin[:, 6:7], fin[:, 5:6], -1.0,
        mkS(chartV, 4, n * BLK2 + 68),
        op0=ALU.mult, op1=ALU.subtract)
    nc.sync.dma_start(d["out"][:], fin[:, 6:7])
    es2.close()
    es.close()


# ============================================================== host wrapper
_PROG_CACHE = {}


def _get_program(cfg: Cfg):
    key = (cfg.n, cfg.v_loc, cfg.n_cores)
    if key not in _PROG_CACHE:
        _PROG_CACHE[key] = build_program(cfg)
    return _PROG_CACHE[key]


def make_inmaps(cfg: Cfg, inputs):
    """Host-side shard/pack of FULL inputs -> per-core DRAM input dicts."""
    x = np.asarray(inputs["x"])
    check_functor_tables(np.asarray(inputs["l_functors"]),
                         np.asarray(inputs["r_functors"]))
    nt_emb = np.asarray(inputs["nt_emb"], np.float32)          # [C, D]
    vocab_W = np.asarray(inputs["vocab_W"], np.float32)        # [D, V]
    vocab_b = np.asarray(inputs["vocab_b"], np.float32)        # [V]

    import ml_dtypes
    bf16 = ml_dtypes.bfloat16

    ntembT = np.zeros((65, CP), np.float32)
    ntembT[0:64, 0:C] = nt_emb.T
    ntembT[64, :] = 1.0
    ntembT = ntembT.astype(bf16)

    mlpW = np.zeros((64, 322), np.float32)
    for j, k in enumerate(("sW1", "r1W1", "r1W2", "r2W1", "r2W2")):
        mlpW[:, j * 64:(j + 1) * 64] = np.asarray(inputs[k], np.float32)
    mlpW[:, 320:322] = np.asarray(inputs["sW2"], np.float32)
    mlpW = mlpW.astype(bf16)

    mlpB = np.zeros((64, 8), np.float32)
    for j, k in enumerate(("sb1", "r1b1", "r1b2", "r2b1", "r2b2")):
        mlpB[:, j] = np.asarray(inputs[k], np.float32)

    ruleWb = np.zeros((36, 144), np.float32)
    ruleWb[:, 0:72] = np.asarray(inputs["rule_W"], np.float32)
    ruleWb[:, 72:144] = np.tile(
        np.asarray(inputs["rule_b"], np.float32)[None, :], (36, 1))

    smallv = np.zeros((1, 16), np.float32)
    smallv[0, 0:2] = np.asarray(inputs["sb2"], np.float32)
    smallv[0, 2:6] = np.asarray(inputs["root_W"], np.float32)[0, 0:4]
    smallv[0, 6:10] = np.asarray(inputs["root_b"], np.float32)[0:4]

    f8 = ml_dtypes.float8_e4m3

    ntembT8 = np.zeros((33, 2 * CP), np.float32)
    full = np.zeros((66, CP), np.float32)
    full[0:64, 0:C] = nt_emb.T
    full[64, :] = 1.0
    ntembT8[:, 0:CP] = full[0:33]
    ntembT8[:, CP:] = full[33:66]
    ntembT8 = np.clip(ntembT8, -448, 448).astype(f8)

    vs = cfg.v_loc
    in_maps = []
    for core in range(cfg.n_cores):
        vocabW = np.zeros((66, cfg.v_pad), np.float32)
        vocabW[64, :] = NEGB
        vocabW[0:64, 0:vs] = vocab_W[:, core * vs:(core + 1) * vs]
        vocabW[64, 0:vs] = vocab_b[core * vs:(core + 1) * vs]
        vocabW8 = np.zeros((33, 2 * cfg.v_pad), np.float32)
        vocabW8[:, 0:cfg.v_pad] = vocabW[0:33]
        vocabW8[:, cfg.v_pad:] = vocabW[33:66]
        vocabW8 = np.clip(vocabW8, -448, 448).astype(f8)

        words = x[core * BLOC:(core + 1) * BLOC, 0:cfg.n]   # [BLOC, n]
        wid = words.T.reshape(-1)                           # pair = i*4 + b
        wordW = np.zeros((65, cfg.pairs), np.float32)
        wordW[0:64, :] = vocab_W[:, wid]
        wordW[64, :] = vocab_b[wid]
        wordW = wordW.astype(bf16)

        in_maps.append({
            "ntembT": ntembT, "ntembT8": ntembT8, "vocabW8": vocabW8,
            "wordW": wordW,
            "mlpW": mlpW, "mlpB": mlpB, "ruleWb": ruleWb, "smallv": smallv,
        })
    return in_maps


def kernel(**inputs) -> np.ndarray:
    cfg = Cfg(n=32, v_loc=V // NCORES, n_cores=NCORES)
    nc = _get_program(cfg)
    in_maps = make_inmaps(cfg, inputs)
    res = bass_utils.run_bass_kernel_spmd(
        nc, in_maps, core_ids=list(range(cfg.n_cores)))
    out = np.concatenate([r["out_nll"].reshape(-1) for r in res.results])
    return out.astype(np.float32)


if __name__ == "__main__":
    from reference import setup_inputs, reference
    inputs = {k: np.asarray(v) for k, v in setup_inputs().items()}
    got = kernel(**inputs)
    exp = np.asarray(reference(**inputs))
    rel = np.max(np.abs(got - exp) / np.maximum(np.abs(exp), 1e-6))
    print("expected:", exp[:8])
    print("got     :", got[:8])
    print("Relative error:", rel)



# revision 63
# speedup vs baseline: 1.0160x; 1.0160x over previous
"""Trainium2 Bass kernel for nn_BasicCGInducer (CKY inside algorithm for a
categorial-grammar inducer).

Strategy (8 NeuronCores):
  - Data-parallel over sentences: core j handles sentences 4j..4j+3.
  - Emission log-partition (the big [C,V] softmax denominator) is
    tensor-parallel over vocab: each core computes sum_v exp(logits) for a
    4000-column V-shard, then one AllReduce of [C] partial sums.
  - Everything else (grammar tables, split-MLP, beta1, CKY) is computed
    per-core on its sentence shard in scaled-exp space (no logsumexp on the
    hot path; per-span running max scales).

kernel(**inputs) takes FULL inputs, shards on host, runs one SPMD bass
program on cores 0-7, and reassembles the [32] output.
"""
import sys
import contextlib

sys.path.insert(0, "/opt/trn_rl_repo")

import numpy as np

import concourse.bass as bass
import concourse.bacc as bacc
import concourse.mybir as mybir
import concourse.tile as tile
from concourse.ap import AP
from concourse import bass_utils

F32 = mybir.dt.float32
F32R = mybir.dt.float32r
BF16 = mybir.dt.bfloat16
I32 = mybir.dt.int32
F8E4 = mybir.dt.float8e4
ALU = mybir.AluOpType
ACTF = mybir.ActivationFunctionType
AXIS = mybir.AxisListType
LN2 = 0.6931471805599453
FE_A = 12102203.161561485           # 2^23 / ln 2
FE_B = float((127 << 23) - 486411)  # Schraudolph bias, rms-centred

# ---------------------------------------------------------------- constants
P4 = 4          # primitive cats
NF = 36         # non-functor cats
C = 2596        # total cats
CP = 2688       # padded C (21 * 128)
NT = CP // 128  # 21 c-tiles
D = 64
B = 32          # total sentences
NCORES = 8
BLOC = B // NCORES  # 4 sentences per core
V = 32000
BLK2 = 72       # per-level block stride in bf16 chart tensors
NEGB = -1.0e5   # bias for padded vocab columns


class Cfg:
    def __init__(self, n=32, v_loc=4000, n_cores=8):
        self.n = n                      # sentence length
        self.v_loc = v_loc              # vocab shard per core
        self.v_pad = ((v_loc + 511) // 512) * 512
        self.n_cores = n_cores
        self.pairs = 4 * n              # (i, b) pairs on partitions


# ------------------------------------------------------------ functor maps
def lf_block_offsets(op):
    """c = off + {A: 4r+a | B: 32r+(a-4) | C: 36(r-4)+a} per derivation of
    the deterministic functor-id tables. op=0 -> l_functors, 1 -> r_functors."""
    return {
        "A": 4 + 16 * op,            # res<4, arg<4 : c = A + 4*res + arg
        "B": 36 + 1280 * op,         # res<4, arg>=4: c = B + 32*res + (arg-4)
        "C": 164 + 1280 * op,        # res>=4      : c = C0 + 36*(res-4) + arg
    }


def check_functor_tables(l_functors, r_functors):
    for op, tab in ((0, l_functors), (1, r_functors)):
        off = lf_block_offsets(op)
        exp = np.zeros((NF, NF), np.int64)  # [arg, res]
        for res in range(NF):
            for arg in range(NF):
                if res < P4 and arg < P4:
                    exp[arg, res] = off["A"] + 4 * res + arg
                elif res < P4:
                    exp[arg, res] = off["B"] + 32 * res + (arg - 4)
                else:
                    exp[arg, res] = off["C"] + 36 * (res - 4) + arg
        assert np.array_equal(np.asarray(tab, np.int64), exp), (
            f"functor table structure mismatch (op={op})")


# ---------------------------------------------------------------- AP helper
def mk(t, parts, off, dims, base_part=0):
    """Raw AP on tile t: partition range [base_part, base_part+parts),
    free offset `off` (elements), extra free dims [[step, count], ...]."""
    w = t.ap[0][0]
    return AP(t.tensor, t.offset + base_part * w + off, [[w, parts]] + dims)


def mkS(t, parts, off, blocks=1, step=72, base_part=0):
    """fp32 view of a pair of bf16 cols at `off` (+k*step) in bf16 tile t."""
    w = t.ap[0][0]
    ap = AP(t.tensor, t.offset + base_part * w + off,
            [[w, parts], [step, blocks], [1, 2]])
    return ap.bitcast(mybir.dt.float32)


# ============================================================ device program
def build_program(cfg: Cfg):
    nc = bacc.Bacc("TRN2", target_bir_lowering=False, debug=False,
                   num_devices=cfg.n_cores)
    d = {
        "ntembT": nc.dram_tensor("ntembT", [65, CP], BF16,
                                 kind="ExternalInput"),
        "ntembT8": nc.dram_tensor("ntembT8", [33, 2 * CP], F8E4,
                                  kind="ExternalInput"),
        "vocabW8": nc.dram_tensor("vocabW8", [33, 2 * cfg.v_pad], F8E4,
                                  kind="ExternalInput"),
        "wordW": nc.dram_tensor("wordW", [65, cfg.pairs], BF16,
                                kind="ExternalInput"),
        "mlpW": nc.dram_tensor("mlpW", [64, 322], BF16, kind="ExternalInput"),
        "mlpB": nc.dram_tensor("mlpB", [64, 8], F32, kind="ExternalInput"),
        "ruleWb": nc.dram_tensor("ruleWb", [36, 144], F32,
                                 kind="ExternalInput"),
        "smallv": nc.dram_tensor("smallv", [1, 16], F32,
                                 kind="ExternalInput"),
        "out": nc.dram_tensor("out_nll", [BLOC, 1], F32,
                              kind="ExternalOutput"),
    }
    with tile.TileContext(nc) as tc:
        _trace(tc, cfg, d)
    nc.compile()
    return nc


def _trace(tc, cfg, d):
    nc = tc.nc
    n, PAIRS, VP = cfg.n, cfg.pairs, cfg.v_pad
    NV = VP // 512                    # 512-col v-tiles per core
    NHALF = (NV + 3) // 4             # ACT chunks of up to 4 v-tiles
    HW = CP // 2                      # MLP half width (1344)

    es = contextlib.ExitStack()
    keep = es.enter_context(tc.tile_pool(name="keep", bufs=1))
    dram = es.enter_context(tc.tile_pool(name="dram", bufs=1, space="DRAM"))

    # ---------------- long-lived tensors
    # chart blocks (bf16 values): 0:36 inside | 36:52 FA | 52:68 FB | pad 4
    chartV = keep.tile([PAIRS, (n + 1) * BLK2], BF16)
    chartEV = keep.tile([PAIRS, (n + 1) * BLK2], BF16)  # end-indexed, rev
    WA = keep.tile([PAIRS, 1300], BF16)   # cols 1296:1298 = M1 (f32 bits)
    WB = keep.tile([PAIRS, 1300], BF16)
    grgl = keep.tile([128, 2592], BF16)   # Gr at 0:1296, Gl at 1296:2592
    M1 = keep.tile([PAIRS, 2], F32)
    mlpB = keep.tile([64, 8], F32)
    smallv = keep.tile([1, 16], F32)
    sumexp_parts = keep.tile([128, NT * NHALF], F32)
    sumexp_loc = keep.tile([128, NT], F32)
    sumexp_g = keep.tile([128, NT], F32)
    s0E = keep.tile([1, NF], F32)
    db = keep.tile([1, 2], F32)
    rsRep = keep.tile([4, 4], F32)
    fin = keep.tile([4, 8], F32)

    nc.sync.dma_start(mlpB[:], d["mlpB"][:])
    nc.sync.dma_start(smallv[:], d["smallv"][:])
    nc.gpsimd.memset(chartV[:], 0.0)
    nc.gpsimd.memset(chartEV[:], 0.0)

    ph1 = contextlib.ExitStack()
    p1 = ph1.enter_context(tc.tile_pool(name="ph1", bufs=1))
    ntembT = p1.tile([65, CP], BF16)
    ntembT8 = p1.tile([33, 2 * CP], F8E4)
    vocabW8 = p1.tile([33, 2 * VP], F8E4)
    wordW = p1.tile([65, PAIRS], BF16)
    mlpW = p1.tile([64, 322], BF16)
    ruleWb = p1.tile([36, 144], F32)
    adjE = p1.tile([1, CP], F32)      # exp-space split1 factor sigmoid(-y)
    zrec_row = p1.tile([1, CP], F32)  # 1/Z per cat, flattened
    E_row = p1.tile([1, CP], F32)     # sigmoid(-y)/Z
    E_bf = p1.tile([1, CP], BF16)
    Erep = p1.tile([PAIRS, CP], BF16)
    zrec21 = p1.tile([128, NT], F32)
    beta1E = p1.tile([PAIRS, CP], BF16)
    ruleflat = p1.tile([1, 36 * 72], F32)

    nc.sync.dma_start(ntembT[:], d["ntembT"][:])
    nc.sync.dma_start(ntembT8[:], d["ntembT8"][:])
    nc.sync.dma_start(vocabW8[:], d["vocabW8"][:])
    nc.sync.dma_start(wordW[:], d["wordW"][:])
    nc.sync.dma_start(mlpW[:], d["mlpW"][:])
    nc.sync.dma_start(ruleWb[:], d["ruleWb"][:])

    # =======================================================================
    # Phase 1: emission partition function (exp in place in PSUM + accum_out)
    # =======================================================================
    # AllReduce is split in two halves of c-tiles: the first is issued as
    # soon as tiles 0..NTH-1 finish, hiding its latency under the tail of
    # the emission loop.
    NTH = 14                 # tiles in cc half 1
    cc_in1 = dram.tile([128, NTH], F32)
    cc_out1 = dram.tile([128, NTH], F32)
    cc_in2 = dram.tile([128, NT - NTH], F32)
    cc_out2 = dram.tile([128, NT - NTH], F32)
    cc_bufs = {0: (cc_in1, cc_out1), NTH: (cc_in2, cc_out2)}
    rg = [list(range(cfg.n_cores))]

    def cc_half(r0, r1):
        ci, co = cc_bufs[r0]
        nc.vector.tensor_reduce(
            sumexp_loc[:, r0:r1],
            mk(sumexp_parts, 128, r0 * NHALF, [[NHALF, r1 - r0], [1, NHALF]]),
            axis=AXIS.X, op=ALU.add)
        nc.sync.dma_start(ci[:], sumexp_loc[:, r0:r1])
        nc.gpsimd.collective_compute(
            "AllReduce", ALU.add, replica_groups=rg,
            ins=[ci[:].opt()], outs=[co[:].opt()])

    with tc.tile_pool(name="psum_e", bufs=2, space="PSUM") as pse, \
         tc.tile_pool(name="scr_e", bufs=2) as scre:
        for ct in range(NT):
            for h in range(NHALF):
                vt0 = h * 4
                nvt = min(4, NV - vt0)
                idx = ct * NHALF + h
                c_lo = vt0 * 512
                c_hi = min((vt0 + nvt) * 512, cfg.v_loc)  # skip pad columns
                pt = pse.tile([128, 512 * nvt], F32, tag="pse")
                for vt in range(nvt):
                    w0 = vt * 512
                    w1 = min((vt + 1) * 512, c_hi - c_lo)
                    if w1 <= w0:
                        continue
                    nc.tensor.matmul(
                        pt[:, w0:w1],
                        mk(ntembT8, 33, ct * 128, [[CP, 2], [1, 128]]),
                        mk(vocabW8, 33, c_lo + w0, [[VP, 2], [1, w1 - w0]]),
                        start=True, stop=True,
                        perf_mode=mybir.MatmulPerfMode.DoubleRow)
                sce = scre.tile([128, 512 * 4], BF16, tag="scre")
                nc.scalar.activation(
                    sce[:, 0:c_hi - c_lo], pt[:, 0:c_hi - c_lo], ACTF.Exp,
                    accum_out=sumexp_parts[:, idx:idx + 1])
            if ct == NTH - 1:
                cc_half(0, NTH)
    cc_half(NTH, NT)

    # =======================================================================
    # Phase 2: split MLP (transposed layout hT [64, *]), rule tables, root
    # (independent of the AllReduce -> overlaps it)
    # =======================================================================
    nc.vector.tensor_tensor(db[:, 0:1], smallv[:, 0:1], smallv[:, 1:2],
                            op=ALU.subtract)

    with tc.tile_pool(name="mlp", bufs=1) as mlp:
        hA = mlp.tile([64, HW], BF16, tag="hA")
        hB = mlp.tile([64, HW], BF16, tag="hB")
        hC = mlp.tile([64, HW], BF16, tag="hC")
        s_rows = mlp.tile([2, HW], F32, tag="srows")
        w1 = mlp.tile([1, HW], F32, tag="w1")
        w2 = mlp.tile([1, HW], F32, tag="w2")
        w3 = mlp.tile([1, HW], F32, tag="w3")

        for half in range(2):
            base = half * HW

            def dense_relu(dst, col0, rhs, bias_col, res_add=None, rb=0,
                           func=ACTF.Relu):
                with tc.tile_pool(name="psum_m", bufs=2,
                                  space="PSUM") as psm:
                    for c0 in range(0, HW, 512):
                        c1 = min(c0 + 512, HW)
                        pm = psm.tile([64, 512], F32, tag="psm")
                        nc.tensor.matmul(pm[:, 0:c1 - c0],
                                         mlpW[:, col0:col0 + 64],
                                         rhs[0:64, rb + c0:rb + c1],
                                         start=True, stop=True)
                        nc.scalar.activation(
                            dst[:, c0:c1], pm[:, 0:c1 - c0], func,
                            bias=mlpB[:, bias_col:bias_col + 1])
                        if res_add is not None:
                            nc.vector.tensor_tensor(
                                dst[:, c0:c1], dst[:, c0:c1],
                                res_add[:, c0:c1], op=ALU.add)

            dense_relu(hA, 0, ntembT, 0, rb=base,
                       func=ACTF.Identity)           # h1 (linear)
            dense_relu(hB, 64, hA, 1)                   # t = relu(h1 W + b)
            dense_relu(hC, 128, hB, 2, res_add=hA)      # h2
            dense_relu(hB, 192, hC, 3)                  # t2
            dense_relu(hA, 256, hB, 4, res_add=hC)      # h3

            with tc.tile_pool(name="psum_s", bufs=2, space="PSUM") as pss:
                for c0 in range(0, HW, 512):
                    c1 = min(c0 + 512, HW)
                    ps = pss.tile([2, 512], F32, tag="pss")
                    nc.tensor.matmul(ps[:, 0:c1 - c0],
                                     mlpW[:, 320:322],
                                     hA[0:64, c0:c1],
                                     start=True, stop=True)
                    nc.vector.tensor_copy(s_rows[:, c0:c1], ps[:, 0:c1 - c0])

            # d = s0 - s1 (s1 via DMA to partition 0)
            nc.sync.dma_start(w1[:], s_rows[1:2, :])
            nc.vector.tensor_tensor(w2[:], s_rows[0:1, :], w1[:],
                                    op=ALU.subtract)
            y = w2
            nc.vector.tensor_scalar_add(y[:], y[:], db[:, 0:1])
            # exp(split1) = exp(-softplus(y)) = sigmoid(-y)
            nc.scalar.activation(adjE[:, base:base + HW], y[:],
                                 ACTF.Sigmoid, scale=-1.0)
            if half == 0:
                # exp(split0) = exp(-softplus(-y)) = sigmoid(y)
                nc.scalar.activation(s0E[:], y[:, 0:NF], ACTF.Sigmoid)

    # rule tables: softmax over 72 per res row
    rsum = keep.tile([36, 72], F32)
    rmax = keep.tile([36, 2], F32)
    rsumexp = keep.tile([36, 2], F32)
    nc.vector.tensor_tensor(rsum[:], ruleWb[:, 0:72], ruleWb[:, 72:144],
                            op=ALU.add)
    nc.vector.tensor_reduce(rmax[:, 0:1], rsum[:], axis=AXIS.X, op=ALU.max)
    nc.vector.tensor_scalar_mul(rmax[:, 1:2], rmax[:, 0:1], -1.0)
    nc.scalar.activation(rsum[:], rsum[:], ACTF.Exp, bias=rmax[:, 1:2],
                         accum_out=rsumexp[:, 0:1])
    nc.vector.reciprocal(rsumexp[:, 1:2], rsumexp[:, 0:1])
    nc.vector.tensor_scalar_mul(rsum[:], rsum[:], rsumexp[:, 1:2])

    # flatten ruleEn to [1, 2592] via DRAM, then G-flats replicated
    rule_d = dram.tile([36, 72], F32)
    nc.sync.dma_start(rule_d[:], rsum[:])
    nc.sync.dma_start(
        AP(ruleflat.tensor, ruleflat.offset,
           [[ruleflat.ap[0][0], 1], [1, 36 * 72]]),
        rule_d[:])
    g_d = dram.tile([2, 1296], BF16)
    gtmp = keep.tile([1, 1296], BF16)
    for row, off in ((0, 0), (1, 36)):   # 0: Gl (larg), 1: Gr (rarg)
        nc.vector.tensor_tensor(
            gtmp[:],
            mk(ruleflat, 1, off, [[72, 36], [1, 36]]),
            mk(s0E, 1, 0, [[1, 36], [0, 36]]),
            op=ALU.mult)
        nc.sync.dma_start(g_d[row:row + 1, :], gtmp[:])
    for goff, row in ((1296, 0), (0, 1)):   # row0=Gl -> cols 1296+, row1=Gr
        nc.sync.dma_start(
            grgl[:, goff:goff + 1296],
            AP(g_d.tensor, g_d.offset + row * g_d.ap[0][0],
               [[0, 128], [1, 1296]]))

    # root: rsEn = softmax(root_W[0,0:4] + root_b[0:4]) replicated to 4 parts
    rs4 = keep.tile([1, 8], F32)
    rsE = keep.tile([1, 8], F32)
    nc.vector.tensor_tensor(rs4[:, 0:4], smallv[:, 2:6], smallv[:, 6:10],
                            op=ALU.add)
    nc.vector.tensor_reduce(rs4[:, 4:5], rs4[:, 0:4], axis=AXIS.X, op=ALU.max)
    nc.vector.tensor_scalar_mul(rs4[:, 5:6], rs4[:, 4:5], -1.0)
    nc.scalar.activation(rsE[:, 0:4], rs4[:, 0:4], ACTF.Exp,
                         bias=rs4[:, 5:6], accum_out=rsE[:, 4:5])
    nc.vector.reciprocal(rsE[:, 5:6], rsE[:, 4:5])
    nc.vector.tensor_scalar_mul(rsE[:, 0:4], rsE[:, 0:4], rsE[:, 5:6])
    rs_d = dram.tile([1, 4], F32)
    nc.sync.dma_start(rs_d[:], rsE[:, 0:4])
    nc.sync.dma_start(rsRep[:],
                      AP(rs_d.tensor, rs_d.offset, [[0, 4], [1, 4]]))

    # =======================================================================
    # Phase 3: beta1 = wordW.T @ ntembT (no adj row; E-factor applied after
    # the AllReduce lands) -> exp tables WA/WB
    # =======================================================================
    with tc.tile_pool(name="psum_b", bufs=1, space="PSUM") as psb:
        pb = psb.tile([PAIRS, CP], F32)
        for c0 in range(0, CP, 512):
            c1 = min(c0 + 512, CP)
            nc.tensor.matmul(pb[:, c0:c1], wordW[:],
                             ntembT[:, c0:c1],
                             start=True, stop=True)
        nc.vector.tensor_reduce(M1[:, 0:1], pb[:, 0:C], axis=AXIS.X,
                                op=ALU.max)
        nc.vector.tensor_scalar_mul(M1[:, 1:2], M1[:, 0:1], -1.0)
        nc.scalar.activation(beta1E[:], pb[:], ACTF.Exp, bias=M1[:, 1:2])

    # ---- AllReduce-dependent tail: E[c] = sigmoid(-y_c) / Z_c, per cc half
    # (half 1 lands while the emission tail is still running)
    z_d = dram.tile([128, NT], F32)
    e_d = dram.tile([1, CP], BF16)
    def e_half(r0, r1):
        c0, c1 = r0 * 128, r1 * 128
        nc.sync.dma_start(sumexp_g[:, r0:r1], cc_bufs[r0][1][:])
        nc.vector.reciprocal(zrec21[:, r0:r1], sumexp_g[:, r0:r1])
        # rearrange [128, r] -> [1, c]  (c = ct*128 + p) via DRAM bounce
        nc.sync.dma_start(z_d[:, r0:r1], zrec21[:, r0:r1])
        nc.sync.dma_start(
            AP(zrec_row.tensor, zrec_row.offset + c0,
               [[zrec_row.ap[0][0], 1], [128, r1 - r0], [1, 128]]),
            AP(z_d.tensor, z_d.offset + r0,
               [[z_d.ap[0][0], 1], [1, r1 - r0], [NT, 128]]))
        nc.vector.tensor_tensor(E_bf[:, c0:c1], adjE[:, c0:c1],
                                zrec_row[:, c0:c1], op=ALU.mult)
        nc.sync.dma_start(e_d[:, c0:c1], E_bf[:, c0:c1])
        nc.sync.dma_start(
            Erep[:, c0:c1],
            AP(e_d.tensor, e_d.offset + c0, [[0, PAIRS], [1, c1 - c0]]))
        nc.vector.tensor_tensor(beta1E[:, c0:c1], beta1E[:, c0:c1],
                                Erep[:, c0:c1], op=ALU.mult)

    def w_block(W, goff, oo, od, io, idm):
        nc.vector.scalar_tensor_tensor(
            mk(W, PAIRS, oo, od), mk(beta1E, PAIRS, io, idm), 1.0,
            mk(grgl, PAIRS, goff + oo, od), op0=ALU.mult, op1=ALU.mult)

    offB = lf_block_offsets(0)   # WB gathers l_functors
    offA = lf_block_offsets(1)   # WA gathers r_functors
    e_half(0, NTH)
    # everything whose beta1E sources lie in cats < NTH*128 = 2176 runs now,
    # hidden under the emission tail / second AllReduce
    w_block(WB, 0, 0, [[36, 4], [1, 4]], offB["A"], [[4, 4], [1, 4]])
    w_block(WB, 0, 4, [[36, 4], [1, 32]], offB["B"], [[32, 4], [1, 32]])
    w_block(WB, 0, 144, [[1, 1152]], offB["C"], [[1, 1152]])
    w_block(WA, 1296, 0, [[36, 4], [1, 4]], offA["A"], [[4, 4], [1, 4]])
    w_block(WA, 1296, 4, [[36, 4], [1, 32]], offA["B"], [[32, 4], [1, 32]])
    cs = NTH * 128 - offA["C"]   # C-block cols with source cat < NTH*128
    w_block(WA, 1296, 144, [[1, cs]], offA["C"], [[1, cs]])
    nc.vector.tensor_copy(mkS(WA, PAIRS, 1296), M1[:, 0:1])
    # chart block L=1 from beta1E (cats < 36, all in half 1)
    nc.vector.tensor_copy(mk(chartV, PAIRS, BLK2, [[1, 36]]), beta1E[:, 0:NF])
    nc.vector.tensor_copy(mkS(chartV, PAIRS, BLK2 + 68), M1[:, 0:1])
    nc.vector.tensor_tensor(
        mk(chartV, PAIRS, BLK2 + 36, [[16, 2], [4, 4], [1, 4]]),
        mk(beta1E, PAIRS, 4, [[16, 2], [4, 4], [1, 4]]),
        mk(grgl, PAIRS, 0, [[1296, 2], [36, 4], [1, 4]]),
        op=ALU.mult)
    # chartEnd block m lives at col (n-m)*BLK2 (reversed layout; makes all
    # k-strided reads positive-step). Block 1: end j = i+1 -> same rows.
    nc.sync.dma_start(mk(chartEV, PAIRS, (n - 1) * BLK2, [[1, BLK2]]),
                      mk(chartV, PAIRS, BLK2, [[1, BLK2]]))
    e_half(NTH, NT)
    w_block(WA, 1296, 144 + cs, [[1, 1152 - cs]],
            offA["C"] + cs, [[1, 1152 - cs]])

    ph1.close()  # free ph1 tensors before the CKY working set

    es2 = contextlib.ExitStack()
    stage_pool = es2.enter_context(tc.tile_pool(name="stage", bufs=4))
    wash_pool = es2.enter_context(tc.tile_pool(name="wash", bufs=4))
    scr = es2.enter_context(tc.tile_pool(name="cky", bufs=4))
    scr1 = es2.enter_context(tc.tile_pool(name="cky1", bufs=3))

    # =======================================================================
    # Phase 4: CKY in scaled-exp space (bf16 values, fp32 scales)
    # chartV[pair, L*BLK2+.]: 0:36 inside | 36:52 FA | 52:68 FB
    # scale (f32 in bf16 cols 68:70). chartEV end-indexed by span end j,
    # block m at col (n-m)*BLK2.
    # =======================================================================
    NI_MAX = max(n - 2, 1)
    for L in range(2, n + 1):
        S = n - L + 1
        PS = 4 * S
        NI = L - 2

        stageV = stage_pool.tile([128, n * BLK2], BF16, tag="stv")
        if L >= 3:   # prefetchable part: blocks 1..L-2
            nc.sync.dma_start(
                mk(stageV, PS, (n - L + 2) * BLK2, [[1, (L - 2) * BLK2]]),
                mk(chartEV, PS, (n - L + 2) * BLK2, [[1, (L - 2) * BLK2]],
                   base_part=4 * (L - 1)))
        # critical part: block L-1 = chartV rows [4 .. 4+PS]; issued on the
        # scalar engine's DMA path so it skips the sync queue backlog
        nc.scalar.dma_start(
            mk(stageV, PS, (n - L + 1) * BLK2, [[1, BLK2]]),
            mk(chartV, PS, (L - 1) * BLK2, [[1, BLK2]], base_part=4))

        wash = wash_pool.tile([128, 1300], BF16, tag="wa")
        nc.sync.dma_start(
            mk(wash, PS, 0, [[1, 1300]]),
            mk(WA, PS, 0, [[1, 1300]], base_part=4 * (L - 1)))

        # ---- edge products (eA/eB scalars factored out so prodA has no
        # DMA or scale dependency and starts the round immediately; the
        # sAsm/mstar/exp chain runs concurrently between the two products).
        # The final round only needs the root's primitive results (res<4).
        RS = 4 if L == n else 36
        prodAB = scr1.tile([128, 2592], BF16, tag="prod")
        nc.vector.tensor_tensor(
            prodAB[0:PS, 0:RS * 36],
            mk(wash, PS, 0, [[1, RS * 36]]),
            mk(chartV, PS, (L - 1) * BLK2, [[0, RS], [1, 36]]),
            op=ALU.mult)
        # ---- scales: sAsm = [sB | sA | sI(k=1..L-1)] -- runs while prodA
        # streams; eAll is only needed after the reduces
        sAsm = scr.tile([128, n + 8], F32, tag="sasm")
        nc.vector.tensor_tensor(
            sAsm[0:PS, 0:1], mkS(stageV, PS, (n - L + 1) * BLK2 + 68),
            M1[0:PS, 0:1], op=ALU.add)
        nc.vector.tensor_tensor(
            sAsm[0:PS, 1:2], mkS(chartV, PS, (L - 1) * BLK2 + 68),
            mkS(wash, PS, 1296), op=ALU.add)
        nc.vector.tensor_tensor(
            sAsm[0:PS, 2:L + 1],
            mkS(chartV, PS, BLK2 + 68, blocks=L - 1),
            mkS(stageV, PS, (n - L + 1) * BLK2 + 68, blocks=L - 1),
            op=ALU.add)
        mstar = scr.tile([128, 2], F32, tag="mstar")
        nc.vector.tensor_reduce(mstar[0:PS, 1:2], sAsm[0:PS, 0:L + 1],
                                axis=AXIS.X, op=ALU.max, negate=True)
        eAll = scr.tile([128, n + 8], F32, tag="eall")
        nc.scalar.activation(eAll[0:PS, 0:L + 1], sAsm[0:PS, 0:L + 1],
                             ACTF.Exp, bias=mstar[0:PS, 1:2])
        nc.vector.tensor_tensor(
            prodAB[0:PS, 1296:1296 + RS * 36],
            mk(WB, PS, 0, [[1, RS * 36]]),
            mk(stageV, PS, (n - L + 1) * BLK2, [[0, RS], [1, 36]]),
            op=ALU.mult)
        redAB = scr.tile([128, 72], F32, tag="red")
        nc.vector.tensor_reduce(redAB[0:PS, 0:RS],
                                mk(prodAB, PS, 0, [[36, RS], [1, 36]]),
                                axis=AXIS.X, op=ALU.add)
        nc.vector.tensor_reduce(redAB[0:PS, RS:2 * RS],
                                mk(prodAB, PS, 1296, [[36, RS], [1, 36]]),
                                axis=AXIS.X, op=ALU.add)
        total36 = scr.tile([128, 40], F32, tag="tot")
        nc.vector.tensor_scalar_mul(total36[0:PS, 0:RS],
                                    redAB[0:PS, 0:RS], eAll[0:PS, 1:2])
        nc.vector.scalar_tensor_tensor(
            total36[0:PS, 0:RS], redAB[0:PS, RS:2 * RS], eAll[0:PS, 0:1],
            total36[0:PS, 0:RS], op0=ALU.mult, op1=ALU.add)


        # ---- interior terms (res<4), batched over k, eI pre-folded in args
        if NI > 0:
            argsI = scr.tile([128, 8 * NI_MAX], BF16, tag="argsi")
            nc.vector.tensor_tensor(   # left args (chart k) x eI[k]
                mk(argsI, PS, 0, [[4, NI], [1, 4]]),
                mk(chartV, PS, BLK2, [[BLK2, NI], [1, 4]]),
                mk(eAll, PS, 2, [[1, NI], [0, 4]]), op=ALU.mult)
            nc.vector.tensor_tensor(   # right args (stage L-k) x eI[k]
                mk(argsI, PS, 4 * NI_MAX, [[4, NI], [1, 4]]),
                mk(stageV, PS, (n - L + 2) * BLK2, [[BLK2, NI], [1, 4]]),
                mk(eAll, PS, 3, [[1, NI], [0, 4]]), op=ALU.mult)
            # tI in k-major 16-blocks (j = 4*res+arg) so the fills read the
            # contiguous FA/FB chart blocks with flat 2-level patterns
            tI = scr1.tile([128, 2 * NI_MAX * 16], BF16, tag="ti")
            nc.vector.tensor_tensor(   # IA: scaled left args x stage FA(L-k)
                mk(tI, PS, 0, [[16, NI], [1, 16]]),
                mk(argsI, PS, 0, [[4, NI], [0, 4], [1, 4]]),
                mk(stageV, PS, (n - L + 1) * BLK2 + 52, [[BLK2, NI], [1, 16]]),
                op=ALU.mult)
            nc.vector.tensor_tensor(   # IB: scaled right args x chart[k] FB
                mk(tI, PS, 16 * NI, [[16, NI], [1, 16]]),
                mk(argsI, PS, 4 * NI_MAX, [[4, NI], [0, 4], [1, 4]]),
                mk(chartV, PS, 2 * BLK2 + 36, [[BLK2, NI], [1, 16]]),
                op=ALU.mult)
            nc.vector.tensor_reduce(   # sum over (side*k, arg) -> [PS, 4]
                total36[0:PS, 36:40],
                mk(tI, PS, 0, [[4, 4], [16, 2 * NI], [1, 4]]),
                axis=AXIS.XY, op=ALU.add)
            nc.vector.tensor_tensor(total36[0:PS, 0:4], total36[0:PS, 0:4],
                                    total36[0:PS, 36:40], op=ALU.add)

        # ---- rescale by a power of 2 near the max (log2-exponent bit
        # tricks on DVE; keeps Exp as the only scalar-engine table in CKY).
        # Odd rounds skip the renorm: values stay bounded enough for one
        # extra level and the next even round rescales.
        if L % 2 == 0 or L == n:
            mval = scr.tile([128, 8], F32, tag="mval")
            nc.vector.tensor_reduce(mval[0:PS, 0:1], total36[0:PS, 0:RS],
                                    axis=AXIS.X, op=ALU.max)
            nc.vector.tensor_scalar(
                mval[0:PS, 5:6].bitcast(I32), mval[0:PS, 0:1].bitcast(I32),
                0x7F800000, None, op0=ALU.bitwise_and)      # exponent bits
            nc.vector.tensor_scalar(
                mval[0:PS, 6:7].bitcast(I32), mval[0:PS, 5:6].bitcast(I32),
                -1, 0x7F000000, op0=ALU.mult, op1=ALU.add)  # bits 2^(127-e)
            nc.vector.tensor_scalar(
                mval[0:PS, 4:5].bitcast(I32), mval[0:PS, 0:1].bitcast(I32),
                23, None, op0=ALU.logical_shift_right)      # biased exp e
            nc.vector.tensor_scalar(
                mval[0:PS, 3:4], mval[0:PS, 4:5].bitcast(I32),
                127, None, op0=ALU.subtract)                # e - 127 (as f32)
            nc.vector.tensor_scalar_mul(
                mk(chartV, PS, L * BLK2, [[1, RS]]),
                total36[0:PS, 0:RS], mval[0:PS, 6:7])
            nc.vector.scalar_tensor_tensor(
                mkS(chartV, PS, L * BLK2 + 68),
                mval[0:PS, 3:4], LN2, mstar[0:PS, 1:2],
                op0=ALU.mult, op1=ALU.subtract)
        else:
            nc.vector.tensor_copy(
                mk(chartV, PS, L * BLK2, [[1, 36]]), total36[0:PS, 0:36])
            nc.vector.tensor_scalar_mul(
                mkS(chartV, PS, L * BLK2 + 68), mstar[0:PS, 1:2], -1.0)
        if L < n:
            nc.vector.tensor_tensor(
                mk(chartV, PS, L * BLK2 + 36, [[16, 2], [4, 4], [1, 4]]),
                mk(chartV, PS, L * BLK2 + 4, [[16, 2], [4, 4], [1, 4]]),
                mk(grgl, PS, 0, [[1296, 2], [36, 4], [1, 4]]),
                op=ALU.mult)
        if L < n:   # chartEnd block L at rows (i+L-1)*4+b
            nc.sync.dma_start(
                mk(chartEV, PS, (n - L) * BLK2, [[1, BLK2]],
                   base_part=4 * (L - 1)),
                mk(chartV, PS, L * BLK2, [[1, BLK2]]))

    # =======================================================================
    # Phase 5: root -> nll per sentence
    # =======================================================================
    nc.vector.tensor_tensor(fin[:, 0:4],
                            mk(chartV, 4, n * BLK2, [[1, 4]]),
                            rsRep[:], op=ALU.mult)
    nc.vector.tensor_reduce(fin[:, 4:5], fin[:, 0:4], axis=AXIS.X, op=ALU.add)
    nc.scalar.activation(fin[:, 5:6], fin[:, 4:5], ACTF.Ln)
    nc.vector.scalar_tensor_tensor(
        fin[:, 6:7], fin[:, 5:6], -1.0,
        mkS(chartV, 4, n * BLK2 + 68),
        op0=ALU.mult, op1=ALU.subtract)
    nc.sync.dma_start(d["out"][:], fin[:, 6:7])
    es2.close()
    es.close()


# ============================================================== host wrapper
_PROG_CACHE = {}


def _get_program(cfg: Cfg):
    key = (cfg.n, cfg.v_loc, cfg.n_cores)
    if key not in _PROG_CACHE:
        _PROG_CACHE[key] = build_program(cfg)
    return _PROG_CACHE[key]


def make_inmaps(cfg: Cfg, inputs):
    """Host-side shard/pack of FULL inputs -> per-core DRAM input dicts."""
    x = np.asarray(inputs["x"])
    check_functor_tables(np.asarray(inputs["l_functors"]),
                         np.asarray(inputs["r_functors"]))
    nt_emb = np.asarray(inputs["nt_emb"], np.float32)          # [C, D]
    vocab_W = np.asarray(inputs["vocab_W"], np.float32)        # [D, V]
    vocab_b = np.asarray(inputs["vocab_b"], np.float32)        # [V]

    import ml_dtypes
    bf16 = ml_dtypes.bfloat16

    ntembT = np.zeros((65, CP), np.float32)
    ntembT[0:64, 0:C] = nt_emb.T
    ntembT[64, :] = 1.0
    ntembT = ntembT.astype(bf16)

    mlpW = np.zeros((64, 322), np.float32)
    for j, k in enumerate(("sW1", "r1W1", "r1W2", "r2W1", "r2W2")):
        mlpW[:, j * 64:(j + 1) * 64] = np.asarray(inputs[k], np.float32)
    mlpW[:, 320:322] = np.asarray(inputs["sW2"], np.float32)
    mlpW = mlpW.astype(bf16)

    mlpB = np.zeros((64, 8), np.float32)
    for j, k in enumerate(("sb1", "r1b1", "r1b2", "r2b1", "r2b2")):
        mlpB[:, j] = np.asarray(inputs[k], np.float32)

    ruleWb = np.zeros((36, 144), np.float32)
    ruleWb[:, 0:72] = np.asarray(inputs["rule_W"], np.float32)
    ruleWb[:, 72:144] = np.tile(
        np.asarray(inputs["rule_b"], np.float32)[None, :], (36, 1))

    smallv = np.zeros((1, 16), np.float32)
    smallv[0, 0:2] = np.asarray(inputs["sb2"], np.float32)
    smallv[0, 2:6] = np.asarray(inputs["root_W"], np.float32)[0, 0:4]
    smallv[0, 6:10] = np.asarray(inputs["root_b"], np.float32)[0:4]

    f8 = ml_dtypes.float8_e4m3

    ntembT8 = np.zeros((33, 2 * CP), np.float32)
    full = np.zeros((66, CP), np.float32)
    full[0:64, 0:C] = nt_emb.T
    full[64, :] = 1.0
    ntembT8[:, 0:CP] = full[0:33]
    ntembT8[:, CP:] = full[33:66]
    ntembT8 = np.clip(ntembT8, -448, 448).astype(f8)

    vs = cfg.v_loc
    in_maps = []
    for core in range(cfg.n_cores):
        vocabW = np.zeros((66, cfg.v_pad), np.float32)
        vocabW[64, :] = NEGB
        vocabW[0:64, 0:vs] = vocab_W[:, core * vs:(core + 1) * vs]
        vocabW[64, 0:vs] = vocab_b[core * vs:(core + 1) * vs]
        vocabW8 = np.zeros((33, 2 * cfg.v_pad), np.float32)
        vocabW8[:, 0:cfg.v_pad] = vocabW[0:33]
        vocabW8[:, cfg.v_pad:] = vocabW[33:66]
        vocabW8 = np.clip(vocabW8, -448, 448).astype(f8)

        words = x[core * BLOC:(core + 1) * BLOC, 0:cfg.n]   # [BLOC, n]
        wid = words.T.reshape(-1)                           # pair = i*4 + b
        wordW = np.zeros((65, cfg.pairs), np.float32)
        wordW[0:64, :] = vocab_W[:, wid]
        wordW[64, :] = vocab_b[wid]
        wordW = wordW.astype(bf16)

        in_maps.append({
            "ntembT": ntembT, "ntembT8": ntembT8, "vocabW8": vocabW8,
            "wordW": wordW,
            "mlpW": mlpW, "mlpB": mlpB, "ruleWb": ruleWb, "smallv": smallv,
        })
    return in_maps


def kernel(**inputs) -> np.ndarray:
    cfg = Cfg(n=32, v_loc=V // NCORES, n_cores=NCORES)
    nc = _get_program(cfg)
    in_maps = make_inmaps(cfg, inputs)
    res = bass_utils.run_bass_kernel_spmd(
        nc, in_maps, core_ids=list(range(cfg.n_cores)))
    out = np.concatenate([r["out_nll"].reshape(-1) for r in res.results])
    return out.astype(np.float32)


if __name__ == "__main__":
    from reference import setup_inputs, reference
    inputs = {k: np.asarray(v) for k, v in setup_inputs().items()}
    got = kernel(**inputs)
    exp = np.asarray(reference(**inputs))
    rel = np.max(np.abs(got - exp) / np.maximum(np.abs(exp), 1e-6))
    print("expected:", exp[:8])
    print("got     :", got[:8])
    print("Relative error:", rel)

